# revision 38
# baseline (speedup 1.0000x reference)
"""Multi-head attention (SEQ=4096, EMBED=1024, 16 heads, Dh=64) on 8 TRN2
NeuronCores, head-parallel: 2 heads per core. Each core computes its two
heads' UNNORMALIZED attention outputs x^T = sum_t P V (plus the softmax
denominator row via a ones column in V'), ships [65, SEQ] f32 per head;
the host normalizes by D and applies the row-sharded output projection
(the fused all-reduce/gather step) in one sgemm.

Per-core pipeline (one TileContext):
  - hidden^T is pre-transposed/bf16-cast on the HOST and shipped in a
    [128, block, chunk, s] layout (contiguous 8KB DMA runs); q/k/v weights
    are shipped pre-cast bf16, k-weights first so the first projection
    matmul starts as early as possible.
  - softmax exp is split per head across TWO engines with per-head
    [128, 512] score PSUM tiles: head 0's exp runs on ACT
    (exp(32u + 32 ln2)) and head 1's on the DVE via a custom single-pass
    8-stage op ((u+2)u+2)^(2^5) = 2^32 e^{32u} (1 + O(u^3)). The per-head
    split halves the exp latency that gates score-PSUM buffer recycling
    (4 single-bank buffers). Scores arrive pre-scaled u = logits/32 via a
    1/16 fold into each of Wq and Wk; the common 2^32 factor cancels in
    the per-head softmax normalization. P is stored bf16.
  - scores^T per chunk as a row-band tile_position pair (two heads
    co-stream in the PE array, sharing the 128-lane moving-data ingest);
    attn'^T accumulated in PSUM where a ones column in V' makes row 64
    the denominator. Chunks are processed in batches of 2 with the attn
    consumption lagged LAG=4 chunks behind score production GLOBALLY
    (across super boundaries), so every group keeps the steady
    [sc sc at at at at] rhythm and exp latency never stalls the PE.
  - drains: each head's [65, SUP] PSUM tile is copied f32 to SBUF (h0 on
    ACT, h1 on DVE) and DMA'd out per super; no on-device output
    projection.

Notes from measurement (TRN2): the PE ingests ~1 moving element per lane
per cycle at 2.4 GHz regardless of dtype, so fp8 DoubleRow scores gave NO
speedup (the 2 k-tiles double the ingested moving elements) and were
reverted; the kernel is within ~10% of the moving-data ingest floor.
Other dead ends measured here: DMA-transpose for the V' layout is far
slower than PE transposes (+37us); splitting the ht0 DMA across two
queues delays the weight loads behind it (+4us); reordering the exp
engine assignment or sc issue order perturbs the tuned schedule (+1-2us).
"""

import os
import sys

sys.path.insert(0, "/opt/trn_rl_repo")

import numpy as np

SEQ = 4096
EMBED = 1024
HEADS = 16
HD = 64
NCORES = 8
HPC = HEADS // NCORES  # 2 heads per core
EC = EMBED // 128  # 8 e-chunks
SUP = 512  # s-super size
NSUP = SEQ // SUP  # 8
TC = SEQ // 128  # 32 t-chunks
JS = SUP // 128  # 4 s-tiles per super

# q and k are stored at 1/16 scale each: the scores matmul then yields
# (q/16)(k/16) = 0.125*q.k/32 = logits/32, the exp pre-scale u.
QKSCALE = 1.0 / 16.0
EXP_BIAS = 32.0 * float(np.log(2.0))  # exp(32u + 32 ln2) = 2^32 e^x

LAST = None  # BassKernelResults of the most recent run (read by test.py)
_CACHE = {}
_REG = {}


def _register_dve_ops():
    """Register the custom DVE exp op via the documented extension point
    (dve_ops.OPS); idempotent."""
    if _REG:
        return _REG
    import concourse.dve_ops as D
    from concourse.dve_spec import Spec, Src0, Src1, C0, C1, lower, _has_src1
    from concourse.dve_uop import DveOpSpec

    def make(name, spec):
        for op in D.OPS:
            if op.name == name:
                _REG[name] = op
                return
        row = D._CUSTOM_DVE_ROW_BASE + len(D.OPS)
        assert row < 0x20
        D._SUB_OPCODE_FOR_NAME[name] = row
        uops = lower(spec, ver="v3")
        sha = DveOpSpec(
            name=name, opcode=row, uops=uops, rd1_en=_has_src1(spec)
        ).sha("v3")
        op = D.DveOp(name, spec, subdim=False, uops_sha={"v3": sha})
        D.OPS.append(op)
        D.CUSTOM_DVE_SPECS[name] = spec
        _REG[name] = op

    # out = ((u + 2) u + 2)^(2^5) = 2^32 exp(32u) (1 + O(u^3)); s0 = s1 = 2.0
    from concourse.dve_spec import sq

    b = (Src0 + C0) * Src0 + C1
    for _ in range(5):
        b = sq(b)

    def _exp_ref(in0, in1, s0, s1, imm2):
        r = (in0 + s0) * in0 + s1
        for _ in range(5):
            r = r * r
        return r

    make("ANT_EXP32_POLY", Spec(body=b, reference=_exp_ref))
    return _REG


def _build():
    import concourse.bacc as bacc
    import concourse.tile as tile
    from concourse import mybir

    _register_dve_ops()

    f32 = mybir.dt.float32
    bf16 = mybir.dt.bfloat16

    nc = bacc.Bacc("TRN2", debug=False, enable_asserts=False, num_devices=NCORES)

    # host-pre-permuted layouts: contiguous per-partition DMA runs
    hidT = nc.dram_tensor(
        "hid_t", [128, NSUP, EC, SUP], bf16, kind="ExternalInput"
    ).ap()
    wq = nc.dram_tensor("w_q", [128, EC, 128], bf16, kind="ExternalInput").ap()
    wk = nc.dram_tensor("w_k", [128, EC, 128], bf16, kind="ExternalInput").ap()
    wv = nc.dram_tensor("w_v", [128, EC, 128], bf16, kind="ExternalInput").ap()
    bqk = nc.dram_tensor("b_qk", [2, 128], f32, kind="ExternalInput").ap()
    bv = nc.dram_tensor("b_v", [128], f32, kind="ExternalInput").ap()
    identb = nc.dram_tensor("identb", [128, 128], bf16, kind="ExternalInput").ap()
    # unnormalized x^T (+ denominator row 64) per head, f32
    outx = nc.dram_tensor(
        "out_x", [HPC, HD + 1, NSUP, SUP], f32, kind="ExternalOutput"
    ).ap()

    with tile.TileContext(nc) as tc:
        _emit(tc, mybir, hidT, wq, wk, wv, bqk, bv, identb, outx)

    nc.compile()
    return nc


def _emit(tc, mybir, hidT, wq, wk, wv, bqk, bv, identb, outx):
    import contextlib

    import concourse.bass as bass

    nc = tc.nc
    ts = bass.ts
    f32 = mybir.dt.float32
    bf16 = mybir.dt.bfloat16
    Exp = mybir.ActivationFunctionType.Exp
    AOT = mybir.AluOpType
    EXP_OP = _REG["ANT_EXP32_POLY"]

    st_ = contextlib.ExitStack()
    persist = st_.enter_context(tc.tile_pool(name="persist", bufs=1))
    hTa = persist.tile([128, EC, SEQ], bf16, tag="hTa")  # hidden^T, all chunks
    qT = persist.tile([128, SEQ], bf16, tag="qT")  # [(h,d), s], 1/16-scaled
    kT = persist.tile([128, SEQ], bf16, tag="kT")
    vP = persist.tile([128, TC, HPC * (HD + 1)], bf16, tag="vP")  # V' + ones col
    wq_sb = persist.tile([128, EC, 128], bf16, tag="wq")
    wk_sb = persist.tile([128, EC, 128], bf16, tag="wk")
    wv_sb = persist.tile([128, EC, 128], bf16, tag="wv")
    idb_sb = persist.tile([128, 128], bf16, tag="idb")
    bq_sb = persist.tile([128, 1], f32, tag="bq")
    bk_sb = persist.tile([128, 1], f32, tag="bk")
    bv_sb = persist.tile([128, 1], f32, tag="bv")
    eb_sb = persist.tile([128, 1], f32, tag="expbias")
    nc.vector.memset(eb_sb, EXP_BIAS)

    def ht_block(b, eng):
        eng.dma_start(out=hTa[:, :, ts(b, SUP)], in_=hidT[:, b])

    # block 0 ships per e-chunk so k_part(0)'s first matmul only waits on
    # the first 128KB; wk first on the act queue
    nc.scalar.dma_start(out=wk_sb, in_=wk)
    nc.scalar.dma_start(out=bk_sb, in_=bqk[1:2, :].rearrange("a p -> p a"))
    for c in range(EC):
        nc.sync.dma_start(out=hTa[:, c, ts(0, SUP)], in_=hidT[:, 0, c])
    ht_block(1, nc.sync)
    nc.scalar.dma_start(out=wq_sb, in_=wq)
    nc.scalar.dma_start(out=bq_sb, in_=bqk[0:1, :].rearrange("a p -> p a"))
    nc.scalar.dma_start(out=wv_sb, in_=wv)
    bv_col = bass.AP(tensor=bv.tensor, offset=bv.offset, ap=[[1, 128], [1, 1]])
    nc.scalar.dma_start(out=bv_sb, in_=bv_col)
    nc.scalar.dma_start(out=idb_sb, in_=identb)

    with tc.tile_pool(name="onesp", bufs=1) as ones_p:
        # ones columns of V' (free positions h*65+64); V fills cols 0..63
        ones_sb = ones_p.tile([128, 1], f32, tag="ones")
        nc.vector.memset(ones_sb, 1.0)
        vP_ones = vP.rearrange("p c (h e) -> p c h e", h=2)[:, :, :, HD : HD + 1]
        ones_b = bass.AP(
            tensor=ones_sb.tensor,
            offset=ones_sb.offset,
            ap=[ones_sb.ap[0], [0, TC], [0, 2], [0, 1]],
        )
        nc.vector.tensor_copy(out=vP_ones, in_=ones_b)

    pT_p = st_.enter_context(tc.tile_pool(name="pT", bufs=12))
    vT_p = st_.enter_context(tc.tile_pool(name="vT", bufs=2))
    xs_p = st_.enter_context(tc.tile_pool(name="xs", bufs=4))
    sc_ps_p = st_.enter_context(tc.tile_pool(name="ps_sc", bufs=4, space="PSUM"))
    at_ps_p = st_.enter_context(tc.tile_pool(name="ps_at", bufs=1, space="PSUM"))
    aux_ps_p = st_.enter_context(tc.tile_pool(name="ps_aux", bufs=2, space="PSUM"))

    if True:
        pT_of = {}
        at_of = {}

        def sc_pair(sup, c):
            # per-head score PSUM tiles + per-head exp (h0 on ACT, h1 on
            # DVE): halves the exp latency gating PSUM buffer recycling.
            sc_h = [
                sc_ps_p.tile([128, SUP], f32, tag="sc", name=f"sc{sup}_{c}_{h}")
                for h in range(HPC)
            ]
            for h in range(HPC):
                nc.tensor.matmul(
                    sc_h[h],
                    kT[ts(h, HD), ts(c, 128)],
                    qT[ts(h, HD), ts(sup, SUP)],
                    start=True,
                    stop=True,
                    tile_position=(h * HD, 0),
                )
            for h in range(HPC):
                pT = pT_p.tile([128, SUP], bf16, tag="pT", name=f"pT{sup}_{c}_{h}")
                if h == 1:
                    nc.vector._custom_dve(
                        EXP_OP, out=pT, in0=sc_h[h], s0=2.0, s1=2.0
                    )
                else:
                    nc.scalar.activation(
                        out=pT, in_=sc_h[h], func=Exp, scale=32.0, bias=eb_sb
                    )
                pT_of[(sup, c, h)] = pT

        def new_at(sup):
            at_of[sup] = [
                at_ps_p.tile([HD + 1, SUP], f32, tag=f"at{h}", name=f"at{sup}_{h}")
                for h in range(HPC)
            ]

        def at_pair(sup, c):
            for h in range(HPC):
                pT = pT_of.pop((sup, c, h))
                nc.tensor.matmul(
                    at_of[sup][h],
                    vP[:, c, ts(h, HD + 1)],
                    pT,
                    start=(c == 0),
                    stop=(c == TC - 1),
                )

        def k_part(b):
            k_ps = aux_ps_p.tile([128, SUP], f32, tag="aux", name=f"k_ps{b}")
            for c in range(EC):
                nc.tensor.matmul(
                    k_ps,
                    wk_sb[:, c, :],
                    hTa[:, c, ts(b, SUP)],
                    start=(c == 0),
                    stop=(c == EC - 1),
                )
            nc.vector.tensor_scalar(
                out=kT[:, ts(b, SUP)], in0=k_ps, scalar1=bk_sb, scalar2=None,
                op0=AOT.add,
            )

        def q_proj(sup):
            q_ps = aux_ps_p.tile([128, SUP], f32, tag="aux", name=f"q_ps{sup}")
            for c in range(EC):
                nc.tensor.matmul(
                    q_ps,
                    wq_sb[:, c, :],
                    hTa[:, c, ts(sup, SUP)],
                    start=(c == 0),
                    stop=(c == EC - 1),
                )
            nc.vector.tensor_scalar(
                out=qT[:, ts(sup, SUP)], in0=q_ps, scalar1=bq_sb, scalar2=None,
                op0=AOT.add,
            )

        def v_part(b):
            # V^T via one K=1024 chain, then PE transposes into V' layout
            vT_ps = aux_ps_p.tile([128, SUP], f32, tag="aux", name=f"vT_ps{b}")
            for c in range(EC):
                nc.tensor.matmul(
                    vT_ps,
                    wv_sb[:, c, :],
                    hTa[:, c, ts(b, SUP)],
                    start=(c == 0),
                    stop=(c == EC - 1),
                )
            vT_sb = vT_p.tile([128, SUP], bf16, tag="vT", name=f"vT{b}")
            nc.vector.tensor_scalar(
                out=vT_sb, in0=vT_ps, scalar1=bv_sb, scalar2=None, op0=AOT.add
            )
            tp_ps = aux_ps_p.tile([128, JS, 128], bf16, tag="aux", name=f"tp_ps{b}")
            for j in range(JS):
                nc.tensor.transpose(tp_ps[:, j, :], vT_sb[:, ts(j, 128)], idb_sb)
            for j in range(JS):
                t_idx = JS * b + j
                dst = vP[:, t_idx, :].rearrange("p (h e) -> p h e", h=2)[:, :, 0:HD]
                nc.vector.tensor_copy(
                    out=dst,
                    in_=tp_ps[:, j, :].rearrange("p (h d) -> p h d", h=2),
                )

        def at_last(sup):
            # final chunk: drain each head right after its last accumulation
            # (f32 PSUM -> SBUF -> DRAM; the host normalizes and projects).
            # h0 drains on ACT, h1 on DVE so the two run in parallel.
            for h in range(HPC):
                pT = pT_of.pop((sup, TC - 1, h))
                nc.tensor.matmul(
                    at_of[sup][h],
                    vP[:, TC - 1, ts(h, HD + 1)],
                    pT,
                    start=False,
                    stop=True,
                )
                xs = xs_p.tile([HD + 1, SUP], f32, tag="xs", name=f"xs{sup}_{h}")
                if h == 0:
                    nc.scalar.copy(out=xs, in_=at_of[sup][h])
                else:
                    nc.vector.tensor_copy(out=xs, in_=at_of[sup][h])
                eng = nc.sync if h == 0 else nc.scalar
                eng.dma_start(out=outx[h, :, sup, :], in_=xs)

        LAG = 4  # attn consumes exp output LAG chunks later (hides exp latency)
        # global lag queue: the attn consumption trails the score production
        # by LAG chunks ACROSS super boundaries, so every group keeps the
        # steady [sc sc at at at at] rhythm (no pure-at tail followed by a
        # buffer-starved run of score pairs at each boundary)
        lagq = []

        def consume_one():
            sup, cc = lagq.pop(0)
            if cc == TC - 1:
                at_last(sup)
            else:
                at_pair(sup, cc)

        def push_group(sup, c):
            # two score pairs back-to-back, then four attn matmuls:
            # fewer weight-buffer conflicts at the pair boundaries
            sc_pair(sup, c)
            sc_pair(sup, c + 1)
            lagq.append((sup, c))
            lagq.append((sup, c + 1))
            while len(lagq) > LAG:
                consume_one()

        # ---- super 0 with phase A interleaved ----------------------------
        new_at(0)
        k_part(0)
        q_proj(0)
        push_group(0, 0)
        v_part(0)
        for b in range(1, NSUP):
            if b + 1 < NSUP:
                ht_block(b + 1, nc.sync)
            for c in range(JS * (b - 1), JS * b, 2):
                if c > 0:
                    push_group(0, c)
            k_part(b)
            v_part(b)
        for c in range(JS * (NSUP - 1), TC, 2):
            push_group(0, c)
        q_proj(1)

        # ---- supers 1..7 -------------------------------------------------
        for sup in range(1, NSUP):
            new_at(sup)
            for c in range(0, TC, 2):
                push_group(sup, c)
                if c == 28 and sup + 1 < NSUP:
                    q_proj(sup + 1)
        while lagq:
            consume_one()

    st_.close()


def _shards(inputs):
    """Host-side prep: per-core input dicts (head-parallel).
    hidden is transposed on the host and shipped bf16; weights pre-cast."""
    import ml_dtypes

    bf16 = ml_dtypes.bfloat16
    hs = np.ascontiguousarray(np.asarray(inputs["hidden_state"], np.float32))
    # [128(p), NSUP, EC, SUP] with hidT[p,b,c,s'] = hidden[512b+s', 128c+p]
    hidT = np.ascontiguousarray(
        hs.T.reshape(EC, 128, NSUP, SUP).transpose(1, 2, 0, 3).astype(bf16)
    )
    Wq = np.asarray(inputs["Wq"], np.float32) * QKSCALE
    bq = np.asarray(inputs["bq"], np.float32) * QKSCALE
    Wk = np.asarray(inputs["Wk"], np.float32) * QKSCALE
    bk = np.asarray(inputs["bk"], np.float32) * QKSCALE
    Wv = np.asarray(inputs["Wv"], np.float32)
    bv = np.asarray(inputs["bv"], np.float32)
    identb = np.eye(128, dtype=bf16)

    in_maps = []
    for c in range(NCORES):
        h0 = HPC * c

        # [H,E,Dh] head-pair -> [E, 2*Dh] -> [128(e), EC, 128(d)] bf16
        def _w(W):
            w = np.transpose(W[h0 : h0 + HPC], (1, 0, 2)).reshape(EMBED, 128)
            return np.ascontiguousarray(
                w.reshape(EC, 128, 128).transpose(1, 0, 2).astype(bf16)
            )

        b_qk = np.stack(
            [bq[h0 : h0 + HPC].reshape(128), bk[h0 : h0 + HPC].reshape(128)]
        )
        b_v = np.ascontiguousarray(bv[h0 : h0 + HPC].reshape(128))
        in_maps.append(
            {
                "hid_t": hidT,
                "w_q": _w(Wq),
                "w_k": _w(Wk),
                "w_v": _w(Wv),
                "b_qk": np.ascontiguousarray(b_qk),
                "b_v": b_v,
                "identb": identb,
            }
        )
    return in_maps


def kernel(**inputs):
    global LAST
    from concourse import bass_utils

    trace = bool(int(os.environ.get("K_TRACE", "0")))
    if trace:
        _install_ntff_shim()

    if "v3" not in _CACHE:
        _CACHE["v3"] = _build()
    nc = _CACHE["v3"]

    in_maps = _shards(inputs)
    res = bass_utils.run_bass_kernel_spmd(
        nc, in_maps, core_ids=list(range(NCORES)), trace=trace
    )
    LAST = res

    # host gather: normalize by the denominator row and apply the
    # row-sharded output projection in one sgemm
    X = np.empty((SEQ, EMBED), np.float32)
    for c in range(NCORES):
        t = res.results[c]["out_x"]  # [2, 65, NSUP, SUP] f32
        x = t[:, 0:HD].reshape(HPC, HD, SEQ)
        dd = t[:, HD].reshape(HPC, 1, SEQ)
        X[:, 128 * c : 128 * (c + 1)] = (x / dd).reshape(128, SEQ).T
    out = X @ np.asarray(inputs["Wo"], np.float32)
    out += np.asarray(inputs["bo"], np.float32)
    return out


def _install_ntff_shim():
    """antenv.axon_hooks is absent from this image; recreate it so
    run_bass_kernel_spmd(trace=True) can reach the NTFF profiling hook."""
    import types

    if "antenv.axon_hooks" in sys.modules:
        return
    try:
        if "/root/.axon_site" not in sys.path:
            sys.path.insert(0, "/root/.axon_site")
        from trn_agent_boot.trn_boot import _ntff_profile_via_ctypes

        hook = _ntff_profile_via_ctypes("/opt/axon/libaxon_pjrt.so")
    except Exception:
        hook = None
    mod = types.ModuleType("antenv.axon_hooks")
    mod._hook = hook
    mod.get_axon_ntff_profile_hook = lambda: mod._hook
    mod.set_axon_ntff_profile_hook = lambda h: setattr(mod, "_hook", h)
    sys.modules["antenv.axon_hooks"] = mod


# revision 40
# speedup vs baseline: 1.0003x; 1.0003x over previous
"""Multi-head attention (SEQ=4096, EMBED=1024, 16 heads, Dh=64) on 8 TRN2
NeuronCores, head-parallel: 2 heads per core. Each core computes its two
heads' UNNORMALIZED attention outputs x^T = sum_t P V (plus the softmax
denominator row via a ones column in V'), ships [65, SEQ] f32 per head;
the host normalizes by D and applies the row-sharded output projection
(the fused all-reduce/gather step) in one sgemm.

Per-core pipeline (one TileContext):
  - hidden^T is pre-transposed/bf16-cast on the HOST and shipped in a
    [128, block, chunk, s] layout (contiguous 8KB DMA runs); q/k/v weights
    are shipped pre-cast bf16, k-weights first so the first projection
    matmul starts as early as possible.
  - softmax exp is split per head across TWO engines with per-head
    [128, 512] score PSUM tiles: head 0's exp runs on ACT
    (exp(32u + 32 ln2)) and head 1's on the DVE via a custom single-pass
    8-stage op ((u+2)u+2)^(2^5) = 2^32 e^{32u} (1 + O(u^3)). The per-head
    split halves the exp latency that gates score-PSUM buffer recycling
    (4 single-bank buffers). Scores arrive pre-scaled u = logits/32 via a
    1/16 fold into each of Wq and Wk; the common 2^32 factor cancels in
    the per-head softmax normalization. P is stored bf16.
  - scores^T per chunk as a row-band tile_position pair (two heads
    co-stream in the PE array, sharing the 128-lane moving-data ingest);
    attn'^T accumulated in PSUM where a ones column in V' makes row 64
    the denominator. Chunks are processed in batches of 2 with the attn
    consumption lagged LAG=4 chunks behind score production GLOBALLY
    (across super boundaries), so every group keeps the steady
    [sc sc at at at at] rhythm and exp latency never stalls the PE.
  - drains: each head's [65, SUP] PSUM tile is copied f32 to SBUF (h0 on
    ACT, h1 on DVE) and DMA'd out per super; no on-device output
    projection.

Notes from measurement (TRN2): the PE ingests ~1 moving element per lane
per cycle at 2.4 GHz regardless of dtype, so fp8 DoubleRow scores gave NO
speedup (the 2 k-tiles double the ingested moving elements) and were
reverted; the kernel is within ~10% of the moving-data ingest floor.
Other dead ends measured here: DMA-transpose for the V' layout is far
slower than PE transposes (+37us); splitting the ht0 DMA across two
queues delays the weight loads behind it (+4us); reordering the exp
engine assignment or sc issue order perturbs the tuned schedule (+1-2us).
"""

import os
import sys

sys.path.insert(0, "/opt/trn_rl_repo")

import numpy as np

SEQ = 4096
EMBED = 1024
HEADS = 16
HD = 64
NCORES = 8
HPC = HEADS // NCORES  # 2 heads per core
EC = EMBED // 128  # 8 e-chunks
SUP = 512  # s-super size
NSUP = SEQ // SUP  # 8
TC = SEQ // 128  # 32 t-chunks
JS = SUP // 128  # 4 s-tiles per super

# q and k are stored at 1/16 scale each: the scores matmul then yields
# (q/16)(k/16) = 0.125*q.k/32 = logits/32, the exp pre-scale u.
QKSCALE = 1.0 / 16.0
EXP_BIAS = 32.0 * float(np.log(2.0))  # exp(32u + 32 ln2) = 2^32 e^x

LAST = None  # BassKernelResults of the most recent run (read by test.py)
_CACHE = {}
_REG = {}


def _register_dve_ops():
    """Register the custom DVE exp op via the documented extension point
    (dve_ops.OPS); idempotent."""
    if _REG:
        return _REG
    import concourse.dve_ops as D
    from concourse.dve_spec import Spec, Src0, Src1, C0, C1, lower, _has_src1
    from concourse.dve_uop import DveOpSpec

    def make(name, spec):
        for op in D.OPS:
            if op.name == name:
                _REG[name] = op
                return
        row = D._CUSTOM_DVE_ROW_BASE + len(D.OPS)
        assert row < 0x20
        D._SUB_OPCODE_FOR_NAME[name] = row
        uops = lower(spec, ver="v3")
        sha = DveOpSpec(
            name=name, opcode=row, uops=uops, rd1_en=_has_src1(spec)
        ).sha("v3")
        op = D.DveOp(name, spec, subdim=False, uops_sha={"v3": sha})
        D.OPS.append(op)
        D.CUSTOM_DVE_SPECS[name] = spec
        _REG[name] = op

    # out = ((u + 1) u + 0.5)^(2^4) = 2^-16 exp(32u) (1 + O(u^3)) with
    # s0 = 1.0, s1 = 0.5: one fewer squaring stage than the older
    # ((u+2)u+2)^32 form (the 2^-16 scale cancels in the per-head softmax
    # normalization; poly error stays well under the accuracy gate)
    from concourse.dve_spec import sq

    b = (Src0 + C0) * Src0 + C1
    for _ in range(4):
        b = sq(b)

    def _exp_ref(in0, in1, s0, s1, imm2):
        r = (in0 + s0) * in0 + s1
        for _ in range(4):
            r = r * r
        return r

    make("ANT_EXP16_POLY", Spec(body=b, reference=_exp_ref))
    return _REG


def _build():
    import concourse.bacc as bacc
    import concourse.tile as tile
    from concourse import mybir

    _register_dve_ops()

    f32 = mybir.dt.float32
    bf16 = mybir.dt.bfloat16

    nc = bacc.Bacc("TRN2", debug=False, enable_asserts=False, num_devices=NCORES)

    # host-pre-permuted layouts: contiguous per-partition DMA runs
    hidT = nc.dram_tensor(
        "hid_t", [128, NSUP, EC, SUP], bf16, kind="ExternalInput"
    ).ap()
    wq = nc.dram_tensor("w_q", [128, EC, 128], bf16, kind="ExternalInput").ap()
    wk = nc.dram_tensor("w_k", [128, EC, 128], bf16, kind="ExternalInput").ap()
    wv = nc.dram_tensor("w_v", [128, EC, 128], bf16, kind="ExternalInput").ap()
    bqk = nc.dram_tensor("b_qk", [2, 128], f32, kind="ExternalInput").ap()
    bv = nc.dram_tensor("b_v", [128], f32, kind="ExternalInput").ap()
    identb = nc.dram_tensor("identb", [128, 128], bf16, kind="ExternalInput").ap()
    # unnormalized x^T (+ denominator row 64) per head, f32
    outx = nc.dram_tensor(
        "out_x", [HPC, HD + 1, NSUP, SUP], f32, kind="ExternalOutput"
    ).ap()

    with tile.TileContext(nc) as tc:
        _emit(tc, mybir, hidT, wq, wk, wv, bqk, bv, identb, outx)

    nc.compile()
    return nc


def _emit(tc, mybir, hidT, wq, wk, wv, bqk, bv, identb, outx):
    import contextlib

    import concourse.bass as bass

    nc = tc.nc
    ts = bass.ts
    f32 = mybir.dt.float32
    bf16 = mybir.dt.bfloat16
    Exp = mybir.ActivationFunctionType.Exp
    AOT = mybir.AluOpType
    EXP_OP = _REG["ANT_EXP16_POLY"]

    st_ = contextlib.ExitStack()
    persist = st_.enter_context(tc.tile_pool(name="persist", bufs=1))
    hTa = persist.tile([128, EC, SEQ], bf16, tag="hTa")  # hidden^T, all chunks
    qT = persist.tile([128, SEQ], bf16, tag="qT")  # [(h,d), s], 1/16-scaled
    kT = persist.tile([128, SEQ], bf16, tag="kT")
    vP = persist.tile([128, TC, HPC * (HD + 1)], bf16, tag="vP")  # V' + ones col
    wq_sb = persist.tile([128, EC, 128], bf16, tag="wq")
    wk_sb = persist.tile([128, EC, 128], bf16, tag="wk")
    wv_sb = persist.tile([128, EC, 128], bf16, tag="wv")
    idb_sb = persist.tile([128, 128], bf16, tag="idb")
    bq_sb = persist.tile([128, 1], f32, tag="bq")
    bk_sb = persist.tile([128, 1], f32, tag="bk")
    bv_sb = persist.tile([128, 1], f32, tag="bv")
    eb_sb = persist.tile([128, 1], f32, tag="expbias")
    nc.vector.memset(eb_sb, EXP_BIAS)

    def ht_block(b, eng):
        eng.dma_start(out=hTa[:, :, ts(b, SUP)], in_=hidT[:, b])

    # block 0 ships per e-chunk so k_part(0)'s first matmul only waits on
    # the first 128KB; wk first on the act queue
    nc.scalar.dma_start(out=wk_sb, in_=wk)
    nc.scalar.dma_start(out=bk_sb, in_=bqk[1:2, :].rearrange("a p -> p a"))
    for c in range(EC):
        nc.sync.dma_start(out=hTa[:, c, ts(0, SUP)], in_=hidT[:, 0, c])
    ht_block(1, nc.sync)
    nc.scalar.dma_start(out=wq_sb, in_=wq)
    nc.scalar.dma_start(out=bq_sb, in_=bqk[0:1, :].rearrange("a p -> p a"))
    nc.scalar.dma_start(out=wv_sb, in_=wv)
    bv_col = bass.AP(tensor=bv.tensor, offset=bv.offset, ap=[[1, 128], [1, 1]])
    nc.scalar.dma_start(out=bv_sb, in_=bv_col)
    nc.scalar.dma_start(out=idb_sb, in_=identb)

    with tc.tile_pool(name="onesp", bufs=1) as ones_p:
        # ones columns of V' (free positions h*65+64); V fills cols 0..63
        ones_sb = ones_p.tile([128, 1], f32, tag="ones")
        nc.vector.memset(ones_sb, 1.0)
        vP_ones = vP.rearrange("p c (h e) -> p c h e", h=2)[:, :, :, HD : HD + 1]
        ones_b = bass.AP(
            tensor=ones_sb.tensor,
            offset=ones_sb.offset,
            ap=[ones_sb.ap[0], [0, TC], [0, 2], [0, 1]],
        )
        nc.vector.tensor_copy(out=vP_ones, in_=ones_b)

    pT_p = st_.enter_context(tc.tile_pool(name="pT", bufs=12))
    vT_p = st_.enter_context(tc.tile_pool(name="vT", bufs=2))
    xs_p = st_.enter_context(tc.tile_pool(name="xs", bufs=4))
    sc_ps_p = st_.enter_context(tc.tile_pool(name="ps_sc", bufs=4, space="PSUM"))
    at_ps_p = st_.enter_context(tc.tile_pool(name="ps_at", bufs=1, space="PSUM"))
    aux_ps_p = st_.enter_context(tc.tile_pool(name="ps_aux", bufs=2, space="PSUM"))

    if True:
        pT_of = {}
        at_of = {}

        def sc_pair(sup, c):
            # per-head score PSUM tiles + per-head exp (h0 on ACT, h1 on
            # DVE): halves the exp latency gating PSUM buffer recycling.
            sc_h = [
                sc_ps_p.tile([128, SUP], f32, tag="sc", name=f"sc{sup}_{c}_{h}")
                for h in range(HPC)
            ]
            for h in range(HPC):
                nc.tensor.matmul(
                    sc_h[h],
                    kT[ts(h, HD), ts(c, 128)],
                    qT[ts(h, HD), ts(sup, SUP)],
                    start=True,
                    stop=True,
                    tile_position=(h * HD, 0),
                )
            for h in range(HPC):
                pT = pT_p.tile([128, SUP], bf16, tag="pT", name=f"pT{sup}_{c}_{h}")
                if h == 1:
                    nc.vector._custom_dve(
                        EXP_OP, out=pT, in0=sc_h[h], s0=1.0, s1=0.5
                    )
                else:
                    nc.scalar.activation(
                        out=pT, in_=sc_h[h], func=Exp, scale=32.0, bias=eb_sb
                    )
                pT_of[(sup, c, h)] = pT

        def new_at(sup):
            at_of[sup] = [
                at_ps_p.tile([HD + 1, SUP], f32, tag=f"at{h}", name=f"at{sup}_{h}")
                for h in range(HPC)
            ]

        def at_pair(sup, c):
            for h in range(HPC):
                pT = pT_of.pop((sup, c, h))
                nc.tensor.matmul(
                    at_of[sup][h],
                    vP[:, c, ts(h, HD + 1)],
                    pT,
                    start=(c == 0),
                    stop=(c == TC - 1),
                )

        def k_part(b):
            k_ps = aux_ps_p.tile([128, SUP], f32, tag="aux", name=f"k_ps{b}")
            for c in range(EC):
                nc.tensor.matmul(
                    k_ps,
                    wk_sb[:, c, :],
                    hTa[:, c, ts(b, SUP)],
                    start=(c == 0),
                    stop=(c == EC - 1),
                )
            nc.vector.tensor_scalar(
                out=kT[:, ts(b, SUP)], in0=k_ps, scalar1=bk_sb, scalar2=None,
                op0=AOT.add,
            )

        def q_proj(sup):
            q_ps = aux_ps_p.tile([128, SUP], f32, tag="aux", name=f"q_ps{sup}")
            for c in range(EC):
                nc.tensor.matmul(
                    q_ps,
                    wq_sb[:, c, :],
                    hTa[:, c, ts(sup, SUP)],
                    start=(c == 0),
                    stop=(c == EC - 1),
                )
            nc.vector.tensor_scalar(
                out=qT[:, ts(sup, SUP)], in0=q_ps, scalar1=bq_sb, scalar2=None,
                op0=AOT.add,
            )

        def v_part(b):
            # V^T via one K=1024 chain, then PE transposes into V' layout
            vT_ps = aux_ps_p.tile([128, SUP], f32, tag="aux", name=f"vT_ps{b}")
            for c in range(EC):
                nc.tensor.matmul(
                    vT_ps,
                    wv_sb[:, c, :],
                    hTa[:, c, ts(b, SUP)],
                    start=(c == 0),
                    stop=(c == EC - 1),
                )
            vT_sb = vT_p.tile([128, SUP], bf16, tag="vT", name=f"vT{b}")
            nc.vector.tensor_scalar(
                out=vT_sb, in0=vT_ps, scalar1=bv_sb, scalar2=None, op0=AOT.add
            )
            tp_ps = aux_ps_p.tile([128, JS, 128], bf16, tag="aux", name=f"tp_ps{b}")
            for j in range(JS):
                nc.tensor.transpose(tp_ps[:, j, :], vT_sb[:, ts(j, 128)], idb_sb)
            for j in range(JS):
                t_idx = JS * b + j
                dst = vP[:, t_idx, :].rearrange("p (h e) -> p h e", h=2)[:, :, 0:HD]
                nc.vector.tensor_copy(
                    out=dst,
                    in_=tp_ps[:, j, :].rearrange("p (h d) -> p h d", h=2),
                )

        def at_last(sup):
            # final chunk: drain each head right after its last accumulation
            # (f32 PSUM -> SBUF -> DRAM; the host normalizes and projects).
            # h0 drains on ACT, h1 on DVE so the two run in parallel.
            for h in range(HPC):
                pT = pT_of.pop((sup, TC - 1, h))
                nc.tensor.matmul(
                    at_of[sup][h],
                    vP[:, TC - 1, ts(h, HD + 1)],
                    pT,
                    start=False,
                    stop=True,
                )
                xs = xs_p.tile([HD + 1, SUP], f32, tag="xs", name=f"xs{sup}_{h}")
                if h == 0:
                    nc.scalar.copy(out=xs, in_=at_of[sup][h])
                else:
                    nc.vector.tensor_copy(out=xs, in_=at_of[sup][h])
                eng = nc.sync if h == 0 else nc.scalar
                eng.dma_start(out=outx[h, :, sup, :], in_=xs)

        LAG = 4  # attn consumes exp output LAG chunks later (hides exp latency)
        # global lag queue: the attn consumption trails the score production
        # by LAG chunks ACROSS super boundaries, so every group keeps the
        # steady [sc sc at at at at] rhythm (no pure-at tail followed by a
        # buffer-starved run of score pairs at each boundary)
        lagq = []

        def consume_one():
            sup, cc = lagq.pop(0)
            if cc == TC - 1:
                at_last(sup)
            else:
                at_pair(sup, cc)

        def push_group(sup, c):
            # two score pairs back-to-back, then four attn matmuls:
            # fewer weight-buffer conflicts at the pair boundaries
            sc_pair(sup, c)
            sc_pair(sup, c + 1)
            lagq.append((sup, c))
            lagq.append((sup, c + 1))
            while len(lagq) > LAG:
                consume_one()

        # ---- super 0 with phase A interleaved ----------------------------
        new_at(0)
        k_part(0)
        q_proj(0)
        push_group(0, 0)
        v_part(0)
        for b in range(1, NSUP):
            if b + 1 < NSUP:
                ht_block(b + 1, nc.sync)
            for c in range(JS * (b - 1), JS * b, 2):
                if c > 0:
                    push_group(0, c)
            k_part(b)
            v_part(b)
        for c in range(JS * (NSUP - 1), TC, 2):
            push_group(0, c)
        q_proj(1)

        # ---- supers 1..7 -------------------------------------------------
        for sup in range(1, NSUP):
            new_at(sup)
            for c in range(0, TC, 2):
                push_group(sup, c)
                if c == 28 and sup + 1 < NSUP:
                    q_proj(sup + 1)
        while lagq:
            consume_one()

    st_.close()


def _shards(inputs):
    """Host-side prep: per-core input dicts (head-parallel).
    hidden is transposed on the host and shipped bf16; weights pre-cast."""
    import ml_dtypes

    bf16 = ml_dtypes.bfloat16
    hs = np.ascontiguousarray(np.asarray(inputs["hidden_state"], np.float32))
    # [128(p), NSUP, EC, SUP] with hidT[p,b,c,s'] = hidden[512b+s', 128c+p]
    hidT = np.ascontiguousarray(
        hs.T.reshape(EC, 128, NSUP, SUP).transpose(1, 2, 0, 3).astype(bf16)
    )
    Wq = np.asarray(inputs["Wq"], np.float32) * QKSCALE
    bq = np.asarray(inputs["bq"], np.float32) * QKSCALE
    Wk = np.asarray(inputs["Wk"], np.float32) * QKSCALE
    bk = np.asarray(inputs["bk"], np.float32) * QKSCALE
    Wv = np.asarray(inputs["Wv"], np.float32)
    bv = np.asarray(inputs["bv"], np.float32)
    identb = np.eye(128, dtype=bf16)

    in_maps = []
    for c in range(NCORES):
        h0 = HPC * c

        # [H,E,Dh] head-pair -> [E, 2*Dh] -> [128(e), EC, 128(d)] bf16
        def _w(W):
            w = np.transpose(W[h0 : h0 + HPC], (1, 0, 2)).reshape(EMBED, 128)
            return np.ascontiguousarray(
                w.reshape(EC, 128, 128).transpose(1, 0, 2).astype(bf16)
            )

        b_qk = np.stack(
            [bq[h0 : h0 + HPC].reshape(128), bk[h0 : h0 + HPC].reshape(128)]
        )
        b_v = np.ascontiguousarray(bv[h0 : h0 + HPC].reshape(128))
        in_maps.append(
            {
                "hid_t": hidT,
                "w_q": _w(Wq),
                "w_k": _w(Wk),
                "w_v": _w(Wv),
                "b_qk": np.ascontiguousarray(b_qk),
                "b_v": b_v,
                "identb": identb,
            }
        )
    return in_maps


def kernel(**inputs):
    global LAST
    from concourse import bass_utils

    trace = bool(int(os.environ.get("K_TRACE", "0")))
    if trace:
        _install_ntff_shim()

    if "v3" not in _CACHE:
        _CACHE["v3"] = _build()
    nc = _CACHE["v3"]

    in_maps = _shards(inputs)
    res = bass_utils.run_bass_kernel_spmd(
        nc, in_maps, core_ids=list(range(NCORES)), trace=trace
    )
    LAST = res

    # host gather: normalize by the denominator row and apply the
    # row-sharded output projection in one sgemm
    X = np.empty((SEQ, EMBED), np.float32)
    for c in range(NCORES):
        t = res.results[c]["out_x"]  # [2, 65, NSUP, SUP] f32
        x = t[:, 0:HD].reshape(HPC, HD, SEQ)
        dd = t[:, HD].reshape(HPC, 1, SEQ)
        X[:, 128 * c : 128 * (c + 1)] = (x / dd).reshape(128, SEQ).T
    out = X @ np.asarray(inputs["Wo"], np.float32)
    out += np.asarray(inputs["bo"], np.float32)
    return out


def _install_ntff_shim():
    """antenv.axon_hooks is absent from this image; recreate it so
    run_bass_kernel_spmd(trace=True) can reach the NTFF profiling hook."""
    import types

    if "antenv.axon_hooks" in sys.modules:
        return
    try:
        if "/root/.axon_site" not in sys.path:
            sys.path.insert(0, "/root/.axon_site")
        from trn_agent_boot.trn_boot import _ntff_profile_via_ctypes

        hook = _ntff_profile_via_ctypes("/opt/axon/libaxon_pjrt.so")
    except Exception:
        hook = None
    mod = types.ModuleType("antenv.axon_hooks")
    mod._hook = hook
    mod.get_axon_ntff_profile_hook = lambda: mod._hook
    mod.set_axon_ntff_profile_hook = lambda h: setattr(mod, "_hook", h)
    sys.modules["antenv.axon_hooks"] = mod


# revision 41
# speedup vs baseline: 1.0033x; 1.0030x over previous
"""Multi-head attention (SEQ=4096, EMBED=1024, 16 heads, Dh=64) on 8 TRN2
NeuronCores, head-parallel: 2 heads per core. Each core computes its two
heads' UNNORMALIZED attention outputs x^T = sum_t P V (plus the softmax
denominator row via a ones column in V'), ships [65, SEQ] f32 per head;
the host normalizes by D and applies the row-sharded output projection
(the fused all-reduce/gather step) in one sgemm.

Per-core pipeline (one TileContext):
  - hidden^T is pre-transposed/bf16-cast on the HOST and shipped in a
    [128, block, chunk, s] layout (contiguous 8KB DMA runs); q/k/v weights
    are shipped pre-cast bf16, k-weights first so the first projection
    matmul starts as early as possible.
  - softmax exp is split per head across TWO engines with per-head
    [128, 512] score PSUM tiles: head 0's exp runs on ACT
    (exp(32u + 32 ln2)) and head 1's on the DVE via a custom single-pass
    8-stage op ((u+2)u+2)^(2^5) = 2^32 e^{32u} (1 + O(u^3)). The per-head
    split halves the exp latency that gates score-PSUM buffer recycling
    (4 single-bank buffers). Scores arrive pre-scaled u = logits/32 via a
    1/16 fold into each of Wq and Wk; the common 2^32 factor cancels in
    the per-head softmax normalization. P is stored bf16.
  - scores^T per chunk as a row-band tile_position pair (two heads
    co-stream in the PE array, sharing the 128-lane moving-data ingest);
    attn'^T accumulated in PSUM where a ones column in V' makes row 64
    the denominator. Chunks are processed in batches of 2 with the attn
    consumption lagged LAG=4 chunks so exp latency never stalls the PE.
  - drains: each head's [65, SUP] PSUM tile is copied f32 to SBUF (h0 on
    ACT, h1 on DVE) and DMA'd out per super; no on-device output
    projection.

Notes from measurement (TRN2): the PE ingests ~1 moving element per lane
per cycle at 2.4 GHz regardless of dtype, so fp8 DoubleRow scores gave NO
speedup (the 2 k-tiles double the ingested moving elements) and were
reverted; the kernel is within ~10% of the moving-data ingest floor.
Other dead ends measured here: DMA-transpose for the V' layout is far
slower than PE transposes (+37us); splitting the ht0 DMA across two
queues delays the weight loads behind it (+4us); reordering the exp
engine assignment or sc issue order perturbs the tuned schedule (+1-2us).
"""

import os
import sys

sys.path.insert(0, "/opt/trn_rl_repo")

import numpy as np

SEQ = 4096
EMBED = 1024
HEADS = 16
HD = 64
NCORES = 8
HPC = HEADS // NCORES  # 2 heads per core
EC = EMBED // 128  # 8 e-chunks
SUP = 512  # s-super size
NSUP = SEQ // SUP  # 8
TC = SEQ // 128  # 32 t-chunks
JS = SUP // 128  # 4 s-tiles per super

# q and k are stored at 1/16 scale each: the scores matmul then yields
# (q/16)(k/16) = 0.125*q.k/32 = logits/32, the exp pre-scale u.
QKSCALE = 1.0 / 16.0
EXP_BIAS = 32.0 * float(np.log(2.0))  # exp(32u + 32 ln2) = 2^32 e^x

LAST = None  # BassKernelResults of the most recent run (read by test.py)
_CACHE = {}
_REG = {}


def _register_dve_ops():
    """Register the custom DVE exp op via the documented extension point
    (dve_ops.OPS); idempotent."""
    if _REG:
        return _REG
    import concourse.dve_ops as D
    from concourse.dve_spec import Spec, Src0, Src1, C0, C1, lower, _has_src1
    from concourse.dve_uop import DveOpSpec

    def make(name, spec):
        for op in D.OPS:
            if op.name == name:
                _REG[name] = op
                return
        row = D._CUSTOM_DVE_ROW_BASE + len(D.OPS)
        assert row < 0x20
        D._SUB_OPCODE_FOR_NAME[name] = row
        uops = lower(spec, ver="v3")
        sha = DveOpSpec(
            name=name, opcode=row, uops=uops, rd1_en=_has_src1(spec)
        ).sha("v3")
        op = D.DveOp(name, spec, subdim=False, uops_sha={"v3": sha})
        D.OPS.append(op)
        D.CUSTOM_DVE_SPECS[name] = spec
        _REG[name] = op

    # out = ((u + 2) u + 2)^(2^5) = 2^32 exp(32u) (1 + O(u^3)); s0 = s1 = 2.0
    from concourse.dve_spec import sq

    b = (Src0 + C0) * Src0 + C1
    for _ in range(5):
        b = sq(b)

    def _exp_ref(in0, in1, s0, s1, imm2):
        r = (in0 + s0) * in0 + s1
        for _ in range(5):
            r = r * r
        return r

    make("ANT_EXP32_POLY", Spec(body=b, reference=_exp_ref))
    return _REG


def _build():
    import concourse.bacc as bacc
    import concourse.tile as tile
    from concourse import mybir

    _register_dve_ops()

    f32 = mybir.dt.float32
    bf16 = mybir.dt.bfloat16

    nc = bacc.Bacc("TRN2", debug=False, enable_asserts=False, num_devices=NCORES)

    # host-pre-permuted layouts: contiguous per-partition DMA runs
    hidT = nc.dram_tensor(
        "hid_t", [128, NSUP, EC, SUP], bf16, kind="ExternalInput"
    ).ap()
    wq = nc.dram_tensor("w_q", [128, EC, 128], bf16, kind="ExternalInput").ap()
    wk = nc.dram_tensor("w_k", [128, EC, 128], bf16, kind="ExternalInput").ap()
    wv = nc.dram_tensor("w_v", [128, EC, 128], bf16, kind="ExternalInput").ap()
    bqk = nc.dram_tensor("b_qk", [2, 128], f32, kind="ExternalInput").ap()
    bv = nc.dram_tensor("b_v", [128], f32, kind="ExternalInput").ap()
    identb = nc.dram_tensor("identb", [128, 128], bf16, kind="ExternalInput").ap()
    # unnormalized x^T (+ denominator row 64) per head, f32
    outx = nc.dram_tensor(
        "out_x", [HPC, HD + 1, NSUP, SUP], f32, kind="ExternalOutput"
    ).ap()

    with tile.TileContext(nc) as tc:
        _emit(tc, mybir, hidT, wq, wk, wv, bqk, bv, identb, outx)

    nc.compile()
    return nc


def _emit(tc, mybir, hidT, wq, wk, wv, bqk, bv, identb, outx):
    import contextlib

    import concourse.bass as bass

    nc = tc.nc
    ts = bass.ts
    f32 = mybir.dt.float32
    bf16 = mybir.dt.bfloat16
    Exp = mybir.ActivationFunctionType.Exp
    AOT = mybir.AluOpType
    EXP_OP = _REG["ANT_EXP32_POLY"]

    st_ = contextlib.ExitStack()
    persist = st_.enter_context(tc.tile_pool(name="persist", bufs=1))
    hTa = persist.tile([128, EC, SEQ], bf16, tag="hTa")  # hidden^T, all chunks
    qT = persist.tile([128, SEQ], bf16, tag="qT")  # [(h,d), s], 1/16-scaled
    kT = persist.tile([128, SEQ], bf16, tag="kT")
    vP = persist.tile([128, TC, HPC * (HD + 1)], bf16, tag="vP")  # V' + ones col
    wq_sb = persist.tile([128, EC, 128], bf16, tag="wq")
    wk_sb = persist.tile([128, EC, 128], bf16, tag="wk")
    wv_sb = persist.tile([128, EC, 128], bf16, tag="wv")
    idb_sb = persist.tile([128, 128], bf16, tag="idb")
    bq_sb = persist.tile([128, 1], f32, tag="bq")
    bk_sb = persist.tile([128, 1], f32, tag="bk")
    bv_sb = persist.tile([128, 1], f32, tag="bv")
    eb_sb = persist.tile([128, 1], f32, tag="expbias")
    nc.vector.memset(eb_sb, EXP_BIAS)

    def ht_block(b, eng):
        eng.dma_start(out=hTa[:, :, ts(b, SUP)], in_=hidT[:, b])

    # block 0 ships per e-chunk so k_part(0)'s first matmul only waits on
    # the first 128KB; wk first on the act queue
    nc.scalar.dma_start(out=wk_sb, in_=wk)
    nc.scalar.dma_start(out=bk_sb, in_=bqk[1:2, :].rearrange("a p -> p a"))
    for c in range(EC):
        nc.sync.dma_start(out=hTa[:, c, ts(0, SUP)], in_=hidT[:, 0, c])
    ht_block(1, nc.sync)
    nc.scalar.dma_start(out=wq_sb, in_=wq)
    nc.scalar.dma_start(out=bq_sb, in_=bqk[0:1, :].rearrange("a p -> p a"))
    nc.scalar.dma_start(out=wv_sb, in_=wv)
    bv_col = bass.AP(tensor=bv.tensor, offset=bv.offset, ap=[[1, 128], [1, 1]])
    nc.scalar.dma_start(out=bv_sb, in_=bv_col)
    nc.scalar.dma_start(out=idb_sb, in_=identb)

    with tc.tile_pool(name="onesp", bufs=1) as ones_p:
        # ones columns of V' (free positions h*65+64); V fills cols 0..63
        ones_sb = ones_p.tile([128, 1], f32, tag="ones")
        nc.vector.memset(ones_sb, 1.0)
        vP_ones = vP.rearrange("p c (h e) -> p c h e", h=2)[:, :, :, HD : HD + 1]
        ones_b = bass.AP(
            tensor=ones_sb.tensor,
            offset=ones_sb.offset,
            ap=[ones_sb.ap[0], [0, TC], [0, 2], [0, 1]],
        )
        nc.vector.tensor_copy(out=vP_ones, in_=ones_b)

    pT_p = st_.enter_context(tc.tile_pool(name="pT", bufs=12))
    vT_p = st_.enter_context(tc.tile_pool(name="vT", bufs=2))
    xs_p = st_.enter_context(tc.tile_pool(name="xs", bufs=4))
    sc_ps_p = st_.enter_context(tc.tile_pool(name="ps_sc", bufs=4, space="PSUM"))
    at_ps_p = st_.enter_context(tc.tile_pool(name="ps_at", bufs=1, space="PSUM"))
    aux_ps_p = st_.enter_context(tc.tile_pool(name="ps_aux", bufs=2, space="PSUM"))

    if True:
        pT_of = {}
        at_of = {}

        def sc_pair(sup, c):
            # per-head score PSUM tiles + per-head exp (h0 on ACT, h1 on
            # DVE): halves the exp latency gating PSUM buffer recycling.
            sc_h = [
                sc_ps_p.tile([128, SUP], f32, tag="sc", name=f"sc{sup}_{c}_{h}")
                for h in range(HPC)
            ]
            for h in range(HPC):
                nc.tensor.matmul(
                    sc_h[h],
                    kT[ts(h, HD), ts(c, 128)],
                    qT[ts(h, HD), ts(sup, SUP)],
                    start=True,
                    stop=True,
                    tile_position=(h * HD, 0),
                )
            for h in range(HPC):
                pT = pT_p.tile([128, SUP], bf16, tag="pT", name=f"pT{sup}_{c}_{h}")
                if h == 1:
                    nc.vector._custom_dve(
                        EXP_OP, out=pT, in0=sc_h[h], s0=2.0, s1=2.0
                    )
                else:
                    nc.scalar.activation(
                        out=pT, in_=sc_h[h], func=Exp, scale=32.0, bias=eb_sb
                    )
                pT_of[(sup, c, h)] = pT

        def new_at(sup):
            at_of[sup] = [
                at_ps_p.tile([HD + 1, SUP], f32, tag=f"at{h}", name=f"at{sup}_{h}")
                for h in range(HPC)
            ]

        def at_pair(sup, c):
            for h in range(HPC):
                pT = pT_of.pop((sup, c, h))
                nc.tensor.matmul(
                    at_of[sup][h],
                    vP[:, c, ts(h, HD + 1)],
                    pT,
                    start=(c == 0),
                    stop=(c == TC - 1),
                )

        def k_part(b):
            k_ps = aux_ps_p.tile([128, SUP], f32, tag="aux", name=f"k_ps{b}")
            for c in range(EC):
                nc.tensor.matmul(
                    k_ps,
                    wk_sb[:, c, :],
                    hTa[:, c, ts(b, SUP)],
                    start=(c == 0),
                    stop=(c == EC - 1),
                )
            nc.vector.tensor_scalar(
                out=kT[:, ts(b, SUP)], in0=k_ps, scalar1=bk_sb, scalar2=None,
                op0=AOT.add,
            )

        def q_proj(sup):
            q_ps = aux_ps_p.tile([128, SUP], f32, tag="aux", name=f"q_ps{sup}")
            for c in range(EC):
                nc.tensor.matmul(
                    q_ps,
                    wq_sb[:, c, :],
                    hTa[:, c, ts(sup, SUP)],
                    start=(c == 0),
                    stop=(c == EC - 1),
                )
            nc.vector.tensor_scalar(
                out=qT[:, ts(sup, SUP)], in0=q_ps, scalar1=bq_sb, scalar2=None,
                op0=AOT.add,
            )

        def v_part(b):
            # V^T via one K=1024 chain, then PE transposes into V' layout
            vT_ps = aux_ps_p.tile([128, SUP], f32, tag="aux", name=f"vT_ps{b}")
            for c in range(EC):
                nc.tensor.matmul(
                    vT_ps,
                    wv_sb[:, c, :],
                    hTa[:, c, ts(b, SUP)],
                    start=(c == 0),
                    stop=(c == EC - 1),
                )
            vT_sb = vT_p.tile([128, SUP], bf16, tag="vT", name=f"vT{b}")
            nc.vector.tensor_scalar(
                out=vT_sb, in0=vT_ps, scalar1=bv_sb, scalar2=None, op0=AOT.add
            )
            tp_ps = aux_ps_p.tile([128, JS, 128], bf16, tag="aux", name=f"tp_ps{b}")
            for j in range(JS):
                nc.tensor.transpose(tp_ps[:, j, :], vT_sb[:, ts(j, 128)], idb_sb)
            for j in range(JS):
                t_idx = JS * b + j
                dst = vP[:, t_idx, :].rearrange("p (h e) -> p h e", h=2)[:, :, 0:HD]
                nc.vector.tensor_copy(
                    out=dst,
                    in_=tp_ps[:, j, :].rearrange("p (h d) -> p h d", h=2),
                )

        def at_last(sup):
            # final chunk: drain each head right after its last accumulation
            # (f32 PSUM -> SBUF -> DRAM; the host normalizes and projects).
            # h0 drains on ACT, h1 on DVE so the two run in parallel.
            for h in range(HPC):
                pT = pT_of.pop((sup, TC - 1, h))
                nc.tensor.matmul(
                    at_of[sup][h],
                    vP[:, TC - 1, ts(h, HD + 1)],
                    pT,
                    start=False,
                    stop=True,
                )
                xs = xs_p.tile([HD + 1, SUP], f32, tag="xs", name=f"xs{sup}_{h}")
                if h == 0:
                    nc.scalar.copy(out=xs, in_=at_of[sup][h])
                else:
                    nc.vector.tensor_copy(out=xs, in_=at_of[sup][h])
                eng = nc.sync if h == 0 else nc.scalar
                eng.dma_start(out=outx[h, :, sup, :], in_=xs)

        LAG = 4  # attn consumes exp output LAG chunks later (hides exp latency)
        # global lag queue: the attn consumption trails the score production
        # by LAG chunks ACROSS super boundaries, so every group keeps the
        # steady [sc sc at at at at] rhythm (no pure-at tail followed by a
        # buffer-starved run of score pairs at each boundary)
        lagq = []

        def consume_one():
            sup, cc = lagq.pop(0)
            if cc == TC - 1:
                at_last(sup)
            else:
                at_pair(sup, cc)

        def push_group(sup, c):
            # two score pairs back-to-back, then four attn matmuls:
            # fewer weight-buffer conflicts at the pair boundaries
            sc_pair(sup, c)
            sc_pair(sup, c + 1)
            lagq.append((sup, c))
            lagq.append((sup, c + 1))
            while len(lagq) > LAG:
                consume_one()

        # ---- super 0 with phase A interleaved ----------------------------
        new_at(0)
        k_part(0)
        q_proj(0)
        push_group(0, 0)
        v_part(0)
        for b in range(1, NSUP):
            if b + 1 < NSUP:
                ht_block(b + 1, nc.sync)
            for c in range(JS * (b - 1), JS * b, 2):
                if c > 0:
                    push_group(0, c)
            k_part(b)
            v_part(b)
        for c in range(JS * (NSUP - 1), TC, 2):
            push_group(0, c)
        q_proj(1)

        # ---- supers 1..7 -------------------------------------------------
        for sup in range(1, NSUP):
            new_at(sup)
            for c in range(0, TC, 2):
                push_group(sup, c)
                if c == 28 and sup + 1 < NSUP:
                    q_proj(sup + 1)
        while lagq:
            consume_one()

    st_.close()


def _shards(inputs):
    """Host-side prep: per-core input dicts (head-parallel).
    hidden is transposed on the host and shipped bf16; weights pre-cast."""
    import ml_dtypes

    bf16 = ml_dtypes.bfloat16
    hs = np.ascontiguousarray(np.asarray(inputs["hidden_state"], np.float32))
    # [128(p), NSUP, EC, SUP] with hidT[p,b,c,s'] = hidden[512b+s', 128c+p]
    hidT = np.ascontiguousarray(
        hs.T.reshape(EC, 128, NSUP, SUP).transpose(1, 2, 0, 3).astype(bf16)
    )
    Wq = np.asarray(inputs["Wq"], np.float32) * QKSCALE
    bq = np.asarray(inputs["bq"], np.float32) * QKSCALE
    Wk = np.asarray(inputs["Wk"], np.float32) * QKSCALE
    bk = np.asarray(inputs["bk"], np.float32) * QKSCALE
    Wv = np.asarray(inputs["Wv"], np.float32)
    bv = np.asarray(inputs["bv"], np.float32)
    identb = np.eye(128, dtype=bf16)

    in_maps = []
    for c in range(NCORES):
        h0 = HPC * c

        # [H,E,Dh] head-pair -> [E, 2*Dh] -> [128(e), EC, 128(d)] bf16
        def _w(W):
            w = np.transpose(W[h0 : h0 + HPC], (1, 0, 2)).reshape(EMBED, 128)
            return np.ascontiguousarray(
                w.reshape(EC, 128, 128).transpose(1, 0, 2).astype(bf16)
            )

        b_qk = np.stack(
            [bq[h0 : h0 + HPC].reshape(128), bk[h0 : h0 + HPC].reshape(128)]
        )
        b_v = np.ascontiguousarray(bv[h0 : h0 + HPC].reshape(128))
        in_maps.append(
            {
                "hid_t": hidT,
                "w_q": _w(Wq),
                "w_k": _w(Wk),
                "w_v": _w(Wv),
                "b_qk": np.ascontiguousarray(b_qk),
                "b_v": b_v,
                "identb": identb,
            }
        )
    return in_maps


def kernel(**inputs):
    global LAST
    from concourse import bass_utils

    trace = bool(int(os.environ.get("K_TRACE", "0")))
    if trace:
        _install_ntff_shim()

    if "v3" not in _CACHE:
        _CACHE["v3"] = _build()
    nc = _CACHE["v3"]

    in_maps = _shards(inputs)
    res = bass_utils.run_bass_kernel_spmd(
        nc, in_maps, core_ids=list(range(NCORES)), trace=trace
    )
    LAST = res

    # host gather: normalize by the denominator row and apply the
    # row-sharded output projection in one sgemm
    X = np.empty((SEQ, EMBED), np.float32)
    for c in range(NCORES):
        t = res.results[c]["out_x"]  # [2, 65, NSUP, SUP] f32
        x = t[:, 0:HD].reshape(HPC, HD, SEQ)
        dd = t[:, HD].reshape(HPC, 1, SEQ)
        X[:, 128 * c : 128 * (c + 1)] = (x / dd).reshape(128, SEQ).T
    out = X @ np.asarray(inputs["Wo"], np.float32)
    out += np.asarray(inputs["bo"], np.float32)
    return out


def _install_ntff_shim():
    """antenv.axon_hooks is absent from this image; recreate it so
    run_bass_kernel_spmd(trace=True) can reach the NTFF profiling hook."""
    import types

    if "antenv.axon_hooks" in sys.modules:
        return
    try:
        if "/root/.axon_site" not in sys.path:
            sys.path.insert(0, "/root/.axon_site")
        from trn_agent_boot.trn_boot import _ntff_profile_via_ctypes

        hook = _ntff_profile_via_ctypes("/opt/axon/libaxon_pjrt.so")
    except Exception:
        hook = None
    mod = types.ModuleType("antenv.axon_hooks")
    mod._hook = hook
    mod.get_axon_ntff_profile_hook = lambda: mod._hook
    mod.set_axon_ntff_profile_hook = lambda h: setattr(mod, "_hook", h)
    sys.modules["antenv.axon_hooks"] = mod
